# revision 2
# baseline (speedup 1.0000x reference)
"""Chunked linear cross-entropy loss on 8 Trainium2 NeuronCores.

Math (per reference):
    logits = hidden @ weight.T           # [N, V]
    logits = 20 * tanh(logits / 20)      # softcap
    lse    = logsumexp(logits, -1)
    nll    = lse - logits[target]
    smooth = lse - logits.mean(-1)
    row    = 0.9 * nll + 0.1 * smooth
    loss   = sum(row * valid)/n_valid + 1e-4 * sum((lse*valid)^2)/n_valid

Sharding: vocab dim V split 8 ways (tensor-parallel). Each core holds
weight rows [c*4096, (c+1)*4096) and the full hidden / targets. Per core
and per token the device computes three partial row-reductions over its
vocab shard:
    esum = sum_v exp(logits_v)     (softcap bounds logits to +-20, so no
                                    running-max is needed: exp stays in
                                    fp32 range)
    csum = sum_v tanh(logits_v/20) (for the label-smoothing mean term)
    xt   = tanh(logit_target/20)   (0 when the target is in another shard)
The host sums partials over cores, takes log for lse, and finishes the
scalar loss in float64.

Device kernel layout per core:
    ht   [128, 16, 4096] bf16  hidden.T  (partition = d%128, block = d//128)
    wt   [128, 16, 4096] bf16  shard of weight.T, same layout
    logits tile = [128 tokens, 512 vocab] accumulated over 16 K-blocks in
    one PSUM bank; ACT does tanh (+row-sum) and exp (+row-sum); DVE does
    the one-hot target gather via iota==target mask + multiply-reduce.
"""

import numpy as np
import ml_dtypes

import concourse.bacc as bacc
import concourse.bass as bass
import concourse.tile as tile
from concourse import mybir
from concourse.bass_utils import run_bass_kernel_spmd

F32 = mybir.dt.float32
BF16 = mybir.dt.bfloat16
AF = mybir.ActivationFunctionType
ALU = mybir.AluOpType

N_CORES = 8
SOFTCAP = 20.0
IGNORE = -100
SMOOTH = 0.1
ZW = 1e-4


def build_nc(n_chunks=32, n_v=8, n_d=16, v_tile=512):
    """One-core SPMD program; identical on all cores, data differs."""
    N = n_chunks * 128
    Vs = n_v * v_tile
    nc = bacc.Bacc("TRN2", target_bir_lowering=False, debug=False)

    ht = nc.dram_tensor("ht", [128, n_d, N], BF16, kind="ExternalInput")
    wt = nc.dram_tensor("wt", [128, n_d, Vs], BF16, kind="ExternalInput")
    tloc = nc.dram_tensor("tloc", [128, n_chunks], F32, kind="ExternalInput")
    iota = nc.dram_tensor("iota", [1, Vs], F32, kind="ExternalInput")
    # osum[:, ch, 0:n_v]       = per-v-tile sum of exp(logits)
    # osum[:, ch, n_v:2n_v]    = per-v-tile sum of tanh(logits/20)
    # osum[:, ch, 2n_v:3n_v]   = per-v-tile target-logit gather (tanh scale)
    osum = nc.dram_tensor("osum", [128, n_chunks, 3 * n_v], F32, kind="ExternalOutput")

    with tile.TileContext(nc) as tc:
        with (
            tc.tile_pool(name="wpool", bufs=1) as wpool,
            tc.tile_pool(name="hpool", bufs=3) as hpool,
            tc.tile_pool(name="cpool", bufs=4) as cpool,
            tc.tile_pool(name="spool", bufs=2) as spool,
            tc.tile_pool(name="mpool", bufs=4) as mpool,
            tc.tile_pool(name="apool", bufs=3) as apool,
            tc.tile_pool(name="onepool", bufs=1) as onepool,
            tc.tile_pool(name="ppool", bufs=8, space="PSUM") as ppool,
        ):
            iota_sb = onepool.tile([128, Vs], F32, tag="iota")
            nc.sync.dma_start(iota_sb[:], iota[:, :].to_broadcast([128, Vs]))
            tl_sb = onepool.tile([128, n_chunks], F32, tag="tloc")
            nc.sync.dma_start(tl_sb[:], tloc[:, :])

            # one tile per vocab column block so early matmuls only wait on
            # their own slice's DMA
            w_tiles = []
            for v in range(n_v):
                wv = wpool.tile([128, n_d, v_tile], BF16, tag=f"w{v}")
                nc.sync.dma_start(wv[:], wt[:, :, v * v_tile : (v + 1) * v_tile])
                w_tiles.append(wv)

            for ch in range(n_chunks):
                hT = hpool.tile([128, n_d, 128], BF16, tag="h")
                nc.sync.dma_start(hT[:], ht[:, :, ch * 128 : (ch + 1) * 128])
                acc = apool.tile([128, 3 * n_v], F32, tag="acc")
                for v in range(n_v):
                    ps = ppool.tile([128, v_tile], F32, tag="ps")
                    for d in range(n_d):
                        nc.tensor.matmul(
                            ps[:],
                            hT[:, d, :],
                            w_tiles[v][:, d, :],
                            start=(d == 0),
                            stop=(d == n_d - 1),
                        )
                    capped = cpool.tile([128, v_tile], F32, tag="capped")
                    nc.scalar.activation(
                        capped[:],
                        ps[:],
                        AF.Tanh,
                        scale=1.0 / SOFTCAP,
                        accum_out=acc[:, n_v + v : n_v + v + 1],
                    )
                    scr = spool.tile([128, v_tile], BF16, tag="scr")
                    nc.scalar.activation(
                        scr[:],
                        capped[:],
                        AF.Exp,
                        scale=SOFTCAP,
                        accum_out=acc[:, v : v + 1],
                    )
                    mask = mpool.tile([128, v_tile], F32, tag="mask")
                    nc.vector.tensor_scalar(
                        out=mask[:],
                        in0=iota_sb[:, v * v_tile : (v + 1) * v_tile],
                        scalar1=tl_sb[:, ch : ch + 1],
                        scalar2=None,
                        op0=ALU.is_equal,
                    )
                    prod = mpool.tile([128, v_tile], F32, tag="prod")
                    nc.vector.tensor_tensor(prod[:], mask[:], capped[:], ALU.mult)
                    nc.vector.tensor_reduce(
                        acc[:, 2 * n_v + v : 2 * n_v + v + 1],
                        prod[:],
                        mybir.AxisListType.X,
                        ALU.add,
                    )
                nc.sync.dma_start(osum[:, ch, :], acc[:])

    nc.compile()
    return nc


def _to_core_layout(mat_t, n_d):
    """[D, X] -> [128, n_d, X] with partition p = d % 128, block = d // 128."""
    D, X = mat_t.shape
    assert D == n_d * 128
    return np.ascontiguousarray(mat_t.reshape(n_d, 128, X).transpose(1, 0, 2))


def prep_inputs(hidden, weight, targets, n_chunks=32, n_v=8, n_d=16, v_tile=512):
    N, D = hidden.shape
    V = weight.shape[0]
    Vs = V // N_CORES
    assert Vs == n_v * v_tile and D == n_d * 128 and N == n_chunks * 128

    ht = _to_core_layout(np.asarray(hidden, np.float32).T.astype(ml_dtypes.bfloat16), n_d)
    iota = np.arange(Vs, dtype=np.float32).reshape(1, Vs)
    t64 = np.asarray(targets, np.int64)

    in_maps = []
    for c in range(N_CORES):
        w_shard = np.asarray(weight[c * Vs : (c + 1) * Vs, :], np.float32)
        wt = _to_core_layout(w_shard.T.astype(ml_dtypes.bfloat16), n_d)
        tloc = (t64 - c * Vs).astype(np.float32).reshape(n_chunks, 128).T
        in_maps.append(
            {
                "ht": ht,
                "wt": wt,
                "tloc": np.ascontiguousarray(tloc),
                "iota": iota,
            }
        )
    return in_maps


def combine(osums, targets, V, n_v=8):
    """osums: list of per-core osum arrays [128, n_chunks, 3*n_v] -> scalar loss."""
    o = np.stack(osums).astype(np.float64)  # [8, 128, nch, 3*n_v]
    esum = o[:, :, :, 0:n_v].sum(axis=(0, 3))          # [128, nch]
    csum = o[:, :, :, n_v : 2 * n_v].sum(axis=(0, 3))
    xt = o[:, :, :, 2 * n_v : 3 * n_v].sum(axis=(0, 3))
    # token t = ch*128 + p  ->  arr[p, ch].T.reshape(-1)
    esum = esum.T.reshape(-1)
    csum = csum.T.reshape(-1)
    xt = xt.T.reshape(-1)

    lse = np.log(esum)
    sum_logits = SOFTCAP * csum
    x_t = SOFTCAP * xt

    t = np.asarray(targets)
    vf = (t != IGNORE).astype(np.float64)
    n_valid = max(vf.sum(), 1.0)
    nll = lse - x_t
    smooth = lse - sum_logits / V
    row = (1.0 - SMOOTH) * nll + SMOOTH * smooth
    loss = (row * vf).sum() / n_valid + ZW * ((lse * vf) ** 2).sum() / n_valid
    return np.asarray(loss, dtype=np.float32)


_NC_CACHE = {}


def get_nc():
    if "nc" not in _NC_CACHE:
        _NC_CACHE["nc"] = build_nc()
    return _NC_CACHE["nc"]


def kernel(hidden, weight, targets):
    nc = get_nc()
    in_maps = prep_inputs(hidden, weight, targets)
    res = run_bass_kernel_spmd(nc, in_maps, core_ids=list(range(N_CORES)))
    return combine([res.results[c]["osum"] for c in range(N_CORES)], targets, weight.shape[0])


# revision 4
# speedup vs baseline: 1.0022x; 1.0022x over previous
"""Chunked linear cross-entropy loss on 8 Trainium2 NeuronCores.

Math (per reference):
    logits = hidden @ weight.T           # [N, V]
    logits = 20 * tanh(logits / 20)      # softcap
    lse    = logsumexp(logits, -1)
    nll    = lse - logits[target]
    smooth = lse - logits.mean(-1)
    row    = 0.9 * nll + 0.1 * smooth
    loss   = sum(row * valid)/n_valid + 1e-4 * sum((lse*valid)^2)/n_valid

Sharding: vocab dim V split 8 ways (tensor-parallel). Each core holds
weight rows [c*4096, (c+1)*4096) and the full hidden / targets. Per core
and per token the device computes three partial row-reductions over its
vocab shard:
    esum = sum_v exp(logits_v)     (softcap bounds logits to +-20, so no
                                    running-max is needed: exp stays in
                                    fp32 range)
    csum = sum_v tanh(logits_v/20) (for the label-smoothing mean term)
    xt   = tanh(logit_target/20)   (0 when the target is in another shard)
The host sums partials over cores, takes log for lse, and finishes the
scalar loss in float64.

Device kernel per core: logits tile = [128 tokens, 512 vocab] accumulated
over the D=2048 contraction in one PSUM bank; ACT does tanh (+row-sum) and
exp (+row-sum); DVE does the one-hot target gather (iota==target mask,
multiply, reduce). Matmul inputs are pre-scaled fp8e4 with DoubleRow (2
fp8 weights per PE cell, K=256 per matmul) by default; bf16 fallback.
"""

import numpy as np
import ml_dtypes

import concourse.bacc as bacc
import concourse.bass as bass
import concourse.tile as tile
from concourse import mybir
from concourse.bass_utils import run_bass_kernel_spmd

F32 = mybir.dt.float32
BF16 = mybir.dt.bfloat16
FP8 = mybir.dt.float8e4
AF = mybir.ActivationFunctionType
ALU = mybir.AluOpType

N_CORES = 8
SOFTCAP = 20.0
IGNORE = -100
SMOOTH = 0.1
ZW = 1e-4

# fp8 pre-scales: keep values well inside TRN e4m3 range (max 240) while
# pushing the small-magnitude tails out of the subnormal region.
H_SCALE = 16.0
W_SCALE = 256.0
FP8_MAX = 240.0

DTYPE = "fp8"  # "fp8" | "bf16"


def build_nc(n_chunks=32, n_v=8, n_d=16, v_tile=512, dtype=DTYPE, timing=False):
    """One-core SPMD program; identical on all cores, data differs.

    timing=True declares ht/wt as Internal DRAM scratch (uninitialized) so
    dispatch overhead — which scales with external-input bytes through the
    axon relay — is minimized; device work is identical.
    """
    N = n_chunks * 128
    Vs = n_v * v_tile
    fp8 = dtype == "fp8"
    n_g = n_d // 2 if fp8 else n_d
    mm_dt = FP8 if fp8 else BF16
    inv_scale = 1.0 / (SOFTCAP * H_SCALE * W_SCALE) if fp8 else 1.0 / SOFTCAP
    perf_mode = mybir.MatmulPerfMode.DoubleRow if fp8 else None

    nc = bacc.Bacc("TRN2", target_bir_lowering=False, debug=False)

    kw = {} if timing else {"kind": "ExternalInput"}
    if fp8:
        ht = nc.dram_tensor("ht", [128, n_g, 2, N], mm_dt, **kw)
        wt = nc.dram_tensor("wt", [128, n_g, 2, Vs], mm_dt, **kw)
    else:
        ht = nc.dram_tensor("ht", [128, n_d, N], mm_dt, **kw)
        wt = nc.dram_tensor("wt", [128, n_d, Vs], mm_dt, **kw)
    tloc = nc.dram_tensor("tloc", [128, n_chunks], F32, kind="ExternalInput")
    iota = nc.dram_tensor("iota", [1, Vs], F32, kind="ExternalInput")
    # osum[:, ch, 0:n_v]       = per-v-tile sum of exp(logits)
    # osum[:, ch, n_v:2n_v]    = per-v-tile sum of tanh(logits/20)
    # osum[:, ch, 2n_v:3n_v]   = per-v-tile target-logit gather (tanh scale)
    osum = nc.dram_tensor("osum", [128, n_chunks, 3 * n_v], F32, kind="ExternalOutput")

    with tile.TileContext(nc) as tc:
        with (
            tc.tile_pool(name="wpool", bufs=1) as wpool,
            tc.tile_pool(name="hpool", bufs=3) as hpool,
            tc.tile_pool(name="cpool", bufs=4) as cpool,
            tc.tile_pool(name="spool", bufs=2) as spool,
            tc.tile_pool(name="mpool", bufs=4) as mpool,
            tc.tile_pool(name="apool", bufs=3) as apool,
            tc.tile_pool(name="onepool", bufs=1) as onepool,
            tc.tile_pool(name="ppool", bufs=8, space="PSUM") as ppool,
        ):
            iota_sb = onepool.tile([128, Vs], F32, tag="iota")
            nc.sync.dma_start(iota_sb[:], iota[:, :].to_broadcast([128, Vs]))
            tl_sb = onepool.tile([128, n_chunks], F32, tag="tloc")
            nc.sync.dma_start(tl_sb[:], tloc[:, :])

            # one tile per vocab column block so early matmuls only wait on
            # their own slice's DMA
            w_tiles = []
            for v in range(n_v):
                if fp8:
                    wv = wpool.tile([128, n_g, 2, v_tile], mm_dt, tag=f"w{v}")
                    nc.sync.dma_start(wv[:], wt[:, :, :, v * v_tile : (v + 1) * v_tile])
                else:
                    wv = wpool.tile([128, n_d, v_tile], mm_dt, tag=f"w{v}")
                    nc.sync.dma_start(wv[:], wt[:, :, v * v_tile : (v + 1) * v_tile])
                w_tiles.append(wv)

            for ch in range(n_chunks):
                if fp8:
                    hT = hpool.tile([128, n_g, 2, 128], mm_dt, tag="h")
                    nc.sync.dma_start(hT[:], ht[:, :, :, ch * 128 : (ch + 1) * 128])
                else:
                    hT = hpool.tile([128, n_d, 128], mm_dt, tag="h")
                    nc.sync.dma_start(hT[:], ht[:, :, ch * 128 : (ch + 1) * 128])
                acc = apool.tile([128, 3 * n_v], F32, tag="acc")
                for v in range(n_v):
                    ps = ppool.tile([128, v_tile], F32, tag="ps")
                    for g in range(n_g):
                        if fp8:
                            nc.tensor.matmul(
                                ps[:],
                                hT[:, g, :, :],
                                w_tiles[v][:, g, :, :],
                                start=(g == 0),
                                stop=(g == n_g - 1),
                                perf_mode=perf_mode,
                            )
                        else:
                            nc.tensor.matmul(
                                ps[:],
                                hT[:, g, :],
                                w_tiles[v][:, g, :],
                                start=(g == 0),
                                stop=(g == n_g - 1),
                            )
                    capped = cpool.tile([128, v_tile], F32, tag="capped")
                    nc.scalar.activation(
                        capped[:],
                        ps[:],
                        AF.Tanh,
                        scale=inv_scale,
                        accum_out=acc[:, n_v + v : n_v + v + 1],
                    )
                    scr = spool.tile([128, v_tile], BF16, tag="scr")
                    nc.scalar.activation(
                        scr[:],
                        capped[:],
                        AF.Exp,
                        scale=SOFTCAP,
                        accum_out=acc[:, v : v + 1],
                    )
                    mask = mpool.tile([128, v_tile], F32, tag="mask")
                    nc.vector.tensor_scalar(
                        out=mask[:],
                        in0=iota_sb[:, v * v_tile : (v + 1) * v_tile],
                        scalar1=tl_sb[:, ch : ch + 1],
                        scalar2=None,
                        op0=ALU.is_equal,
                    )
                    prod = mpool.tile([128, v_tile], F32, tag="prod")
                    nc.vector.tensor_tensor(prod[:], mask[:], capped[:], ALU.mult)
                    nc.vector.tensor_reduce(
                        acc[:, 2 * n_v + v : 2 * n_v + v + 1],
                        prod[:],
                        mybir.AxisListType.X,
                        ALU.add,
                    )
                nc.sync.dma_start(osum[:, ch, :], acc[:])

    nc.compile()
    return nc


def _to_core_layout(mat_t, n_d):
    """[D, X] f32 -> bf16 [128, n_d, X] with partition p = d % 128."""
    D, X = mat_t.shape
    assert D == n_d * 128
    return np.ascontiguousarray(
        mat_t.astype(ml_dtypes.bfloat16).reshape(n_d, 128, X).transpose(1, 0, 2)
    )


def _to_core_layout_fp8(mat_t, n_g, scale):
    """[D, X] f32 -> fp8e4 [128, n_g, 2, X]; d = g*256 + j*128 + ki."""
    D, X = mat_t.shape
    assert D == n_g * 256
    m = np.clip(mat_t * scale, -FP8_MAX, FP8_MAX).astype(ml_dtypes.float8_e4m3)
    return np.ascontiguousarray(m.reshape(n_g, 2, 128, X).transpose(2, 0, 1, 3))


def prep_inputs(
    hidden, weight, targets, n_chunks=32, n_v=8, n_d=16, v_tile=512, dtype=DTYPE
):
    N, D = hidden.shape
    V = weight.shape[0]
    Vs = V // N_CORES
    assert Vs == n_v * v_tile and D == n_d * 128 and N == n_chunks * 128
    fp8 = dtype == "fp8"
    n_g = n_d // 2

    hT = np.asarray(hidden, np.float32).T
    if fp8:
        ht = _to_core_layout_fp8(hT, n_g, H_SCALE)
    else:
        ht = _to_core_layout(hT, n_d)
    iota = np.arange(Vs, dtype=np.float32).reshape(1, Vs)
    t64 = np.asarray(targets, np.int64)

    in_maps = []
    for c in range(N_CORES):
        wT = np.asarray(weight[c * Vs : (c + 1) * Vs, :], np.float32).T
        if fp8:
            wt = _to_core_layout_fp8(wT, n_g, W_SCALE)
        else:
            wt = _to_core_layout(wT, n_d)
        tloc = (t64 - c * Vs).astype(np.float32).reshape(n_chunks, 128).T
        in_maps.append(
            {"ht": ht, "wt": wt, "tloc": np.ascontiguousarray(tloc), "iota": iota}
        )
    return in_maps


def combine(osums, targets, V, n_v=8):
    """osums: list of per-core osum arrays [128, n_chunks, 3*n_v] -> scalar loss."""
    o = np.stack(osums).astype(np.float64)  # [8, 128, nch, 3*n_v]
    esum = o[:, :, :, 0:n_v].sum(axis=(0, 3))  # [128, nch]
    csum = o[:, :, :, n_v : 2 * n_v].sum(axis=(0, 3))
    xt = o[:, :, :, 2 * n_v : 3 * n_v].sum(axis=(0, 3))
    # token t = ch*128 + p  ->  arr[p, ch].T.reshape(-1)
    esum = esum.T.reshape(-1)
    csum = csum.T.reshape(-1)
    xt = xt.T.reshape(-1)

    lse = np.log(esum)
    sum_logits = SOFTCAP * csum
    x_t = SOFTCAP * xt

    t = np.asarray(targets)
    vf = (t != IGNORE).astype(np.float64)
    n_valid = max(vf.sum(), 1.0)
    nll = lse - x_t
    smooth = lse - sum_logits / V
    row = (1.0 - SMOOTH) * nll + SMOOTH * smooth
    loss = (row * vf).sum() / n_valid + ZW * ((lse * vf) ** 2).sum() / n_valid
    return np.asarray(loss, dtype=np.float32)


_NC_CACHE = {}


def get_nc(dtype=DTYPE):
    if dtype not in _NC_CACHE:
        _NC_CACHE[dtype] = build_nc(dtype=dtype)
    return _NC_CACHE[dtype]


def kernel(hidden, weight, targets):
    nc = get_nc()
    in_maps = prep_inputs(hidden, weight, targets)
    res = run_bass_kernel_spmd(nc, in_maps, core_ids=list(range(N_CORES)))
    return combine(
        [res.results[c]["osum"] for c in range(N_CORES)], targets, weight.shape[0]
    )


# revision 6
# speedup vs baseline: 3.2167x; 3.2096x over previous
"""Chunked linear cross-entropy loss on 8 Trainium2 NeuronCores.

Math (per reference):
    logits = hidden @ weight.T           # [N, V]
    logits = 20 * tanh(logits / 20)      # softcap
    lse    = logsumexp(logits, -1)
    nll    = lse - logits[target]
    smooth = lse - logits.mean(-1)
    row    = 0.9 * nll + 0.1 * smooth
    loss   = sum(row * valid)/n_valid + 1e-4 * sum((lse*valid)^2)/n_valid

Sharding: vocab dim V split 8 ways (tensor-parallel). Each core holds
weight rows [c*4096, (c+1)*4096) and the full hidden / targets. Per core
and per token the device computes three partial row-reductions over its
vocab shard:
    esum = sum_v exp(logits_v)     (softcap bounds logits to +-20, so no
                                    running-max is needed: exp stays in
                                    fp32 range)
    csum = sum_v tanh(logits_v/20) (for the label-smoothing mean term)
    xt   = tanh(logit_target/20)   (0 when the target is in another shard)
The host sums partials over cores, takes log for lse, and finishes the
scalar loss in float64.

Device kernel per core: logits tile = [128 tokens, 512 vocab] accumulated
over the D=2048 contraction in one PSUM bank; ACT does tanh (+row-sum) and
exp (+row-sum); DVE does the one-hot target gather (iota==target mask,
multiply, reduce). Matmul inputs are pre-scaled fp8e4 with DoubleRow (2
fp8 weights per PE cell, K=256 per matmul) by default; bf16 fallback.
"""

import numpy as np
import ml_dtypes

import concourse.bacc as bacc
import concourse.bass as bass
import concourse.tile as tile
from concourse import mybir
from concourse.bass_utils import run_bass_kernel_spmd

F32 = mybir.dt.float32
BF16 = mybir.dt.bfloat16
FP8 = mybir.dt.float8e4
AF = mybir.ActivationFunctionType
ALU = mybir.AluOpType

N_CORES = 8
SOFTCAP = 20.0
IGNORE = -100
SMOOTH = 0.1
ZW = 1e-4

# fp8 pre-scales: keep values well inside TRN e4m3 range (max 240) while
# pushing the small-magnitude tails out of the subnormal region.
H_SCALE = 16.0
W_SCALE = 256.0
FP8_MAX = 240.0

DTYPE = "fp8"  # "fp8" | "bf16"


def build_nc(
    n_chunks=32, n_v=8, n_d=16, v_tile=512, dtype=DTYPE, timing=False, n_reps=1
):
    """One-core SPMD program; identical on all cores, data differs.

    timing=True declares ht/wt as Internal DRAM scratch (uninitialized) so
    dispatch overhead — which scales with external-input bytes through the
    axon relay — is minimized; device work is identical. n_reps>1 repeats
    the whole token loop (timing only): device time per rep is isolated by
    differencing wall times of n_reps=1 vs n_reps=K builds, cancelling the
    (noisy, ~100ms) per-dispatch overhead.
    """
    N = n_chunks * 128
    Vs = n_v * v_tile
    fp8 = dtype == "fp8"
    n_g = n_d // 2 if fp8 else n_d
    mm_dt = FP8 if fp8 else BF16
    inv_scale = 1.0 / (SOFTCAP * H_SCALE * W_SCALE) if fp8 else 1.0 / SOFTCAP
    perf_mode = mybir.MatmulPerfMode.DoubleRow if fp8 else None

    nc = bacc.Bacc("TRN2", target_bir_lowering=False, debug=False)

    kw = {} if timing else {"kind": "ExternalInput"}
    if fp8:
        ht = nc.dram_tensor("ht", [128, n_g, 2, N], mm_dt, **kw)
        wt = nc.dram_tensor("wt", [128, n_g, 2, Vs], mm_dt, **kw)
    else:
        ht = nc.dram_tensor("ht", [128, n_d, N], mm_dt, **kw)
        wt = nc.dram_tensor("wt", [128, n_d, Vs], mm_dt, **kw)
    tloc = nc.dram_tensor("tloc", [128, n_chunks], F32, kind="ExternalInput")
    iota = nc.dram_tensor("iota", [1, Vs], F32, kind="ExternalInput")
    # osum[:, ch, 0:n_v]       = per-v-tile sum of exp(logits)
    # osum[:, ch, n_v:2n_v]    = per-v-tile sum of tanh(logits/20)
    # osum[:, ch, 2n_v:3n_v]   = per-v-tile target-logit gather (tanh scale)
    osum = nc.dram_tensor("osum", [128, n_chunks, 3 * n_v], F32, kind="ExternalOutput")

    with tile.TileContext(nc) as tc:
        with (
            tc.tile_pool(name="wpool", bufs=1) as wpool,
            tc.tile_pool(name="hpool", bufs=3) as hpool,
            tc.tile_pool(name="cpool", bufs=4) as cpool,
            tc.tile_pool(name="spool", bufs=2) as spool,
            tc.tile_pool(name="mpool", bufs=4) as mpool,
            tc.tile_pool(name="apool", bufs=3) as apool,
            tc.tile_pool(name="onepool", bufs=1) as onepool,
            tc.tile_pool(name="ppool", bufs=8, space="PSUM") as ppool,
        ):
            iota_sb = onepool.tile([128, Vs], F32, tag="iota")
            nc.sync.dma_start(iota_sb[:], iota[:, :].to_broadcast([128, Vs]))
            tl_sb = onepool.tile([128, n_chunks], F32, tag="tloc")
            nc.sync.dma_start(tl_sb[:], tloc[:, :])

            # one tile per vocab column block so early matmuls only wait on
            # their own slice's DMA
            w_tiles = []
            for v in range(n_v):
                if fp8:
                    wv = wpool.tile([128, n_g, 2, v_tile], mm_dt, tag=f"w{v}")
                    nc.sync.dma_start(wv[:], wt[:, :, :, v * v_tile : (v + 1) * v_tile])
                else:
                    wv = wpool.tile([128, n_d, v_tile], mm_dt, tag=f"w{v}")
                    nc.sync.dma_start(wv[:], wt[:, :, v * v_tile : (v + 1) * v_tile])
                w_tiles.append(wv)

            for ch in range(n_chunks * n_reps):
                ch = ch % n_chunks
                if fp8:
                    hT = hpool.tile([128, n_g, 2, 128], mm_dt, tag="h")
                    nc.sync.dma_start(hT[:], ht[:, :, :, ch * 128 : (ch + 1) * 128])
                else:
                    hT = hpool.tile([128, n_d, 128], mm_dt, tag="h")
                    nc.sync.dma_start(hT[:], ht[:, :, ch * 128 : (ch + 1) * 128])
                acc = apool.tile([128, 3 * n_v], F32, tag="acc")
                for v in range(n_v):
                    ps = ppool.tile([128, v_tile], F32, tag="ps")
                    for g in range(n_g):
                        if fp8:
                            nc.tensor.matmul(
                                ps[:],
                                hT[:, g, :, :],
                                w_tiles[v][:, g, :, :],
                                start=(g == 0),
                                stop=(g == n_g - 1),
                                perf_mode=perf_mode,
                            )
                        else:
                            nc.tensor.matmul(
                                ps[:],
                                hT[:, g, :],
                                w_tiles[v][:, g, :],
                                start=(g == 0),
                                stop=(g == n_g - 1),
                            )
                    capped = cpool.tile([128, v_tile], F32, tag="capped")
                    nc.scalar.activation(
                        capped[:],
                        ps[:],
                        AF.Tanh,
                        scale=inv_scale,
                        accum_out=acc[:, n_v + v : n_v + v + 1],
                    )
                    scr = spool.tile([128, v_tile], BF16, tag="scr")
                    nc.scalar.activation(
                        scr[:],
                        capped[:],
                        AF.Exp,
                        scale=SOFTCAP,
                        accum_out=acc[:, v : v + 1],
                    )
                    mask = mpool.tile([128, v_tile], F32, tag="mask")
                    nc.vector.tensor_scalar(
                        out=mask[:],
                        in0=iota_sb[:, v * v_tile : (v + 1) * v_tile],
                        scalar1=tl_sb[:, ch : ch + 1],
                        scalar2=None,
                        op0=ALU.is_equal,
                    )
                    prod = mpool.tile([128, v_tile], F32, tag="prod")
                    nc.vector.tensor_tensor(prod[:], mask[:], capped[:], ALU.mult)
                    nc.vector.tensor_reduce(
                        acc[:, 2 * n_v + v : 2 * n_v + v + 1],
                        prod[:],
                        mybir.AxisListType.X,
                        ALU.add,
                    )
                nc.sync.dma_start(osum[:, ch, :], acc[:])

    nc.compile()
    return nc


def _to_core_layout(mat_t, n_d):
    """[D, X] f32 -> bf16 [128, n_d, X] with partition p = d % 128."""
    D, X = mat_t.shape
    assert D == n_d * 128
    return np.ascontiguousarray(
        mat_t.astype(ml_dtypes.bfloat16).reshape(n_d, 128, X).transpose(1, 0, 2)
    )


def _to_core_layout_fp8(mat_t, n_g, scale):
    """[D, X] f32 -> fp8e4 [128, n_g, 2, X]; d = g*256 + j*128 + ki."""
    D, X = mat_t.shape
    assert D == n_g * 256
    m = np.clip(mat_t * scale, -FP8_MAX, FP8_MAX).astype(ml_dtypes.float8_e4m3)
    return np.ascontiguousarray(m.reshape(n_g, 2, 128, X).transpose(2, 0, 1, 3))


def prep_inputs(
    hidden, weight, targets, n_chunks=32, n_v=8, n_d=16, v_tile=512, dtype=DTYPE
):
    N, D = hidden.shape
    V = weight.shape[0]
    Vs = V // N_CORES
    assert Vs == n_v * v_tile and D == n_d * 128 and N == n_chunks * 128
    fp8 = dtype == "fp8"
    n_g = n_d // 2

    hT = np.asarray(hidden, np.float32).T
    if fp8:
        ht = _to_core_layout_fp8(hT, n_g, H_SCALE)
    else:
        ht = _to_core_layout(hT, n_d)
    iota = np.arange(Vs, dtype=np.float32).reshape(1, Vs)
    t64 = np.asarray(targets, np.int64)

    in_maps = []
    for c in range(N_CORES):
        wT = np.asarray(weight[c * Vs : (c + 1) * Vs, :], np.float32).T
        if fp8:
            wt = _to_core_layout_fp8(wT, n_g, W_SCALE)
        else:
            wt = _to_core_layout(wT, n_d)
        tloc = (t64 - c * Vs).astype(np.float32).reshape(n_chunks, 128).T
        in_maps.append(
            {"ht": ht, "wt": wt, "tloc": np.ascontiguousarray(tloc), "iota": iota}
        )
    return in_maps


def combine(osums, targets, V, n_v=8):
    """osums: list of per-core osum arrays [128, n_chunks, 3*n_v] -> scalar loss."""
    o = np.stack(osums).astype(np.float64)  # [8, 128, nch, 3*n_v]
    esum = o[:, :, :, 0:n_v].sum(axis=(0, 3))  # [128, nch]
    csum = o[:, :, :, n_v : 2 * n_v].sum(axis=(0, 3))
    xt = o[:, :, :, 2 * n_v : 3 * n_v].sum(axis=(0, 3))
    # token t = ch*128 + p  ->  arr[p, ch].T.reshape(-1)
    esum = esum.T.reshape(-1)
    csum = csum.T.reshape(-1)
    xt = xt.T.reshape(-1)

    lse = np.log(esum)
    sum_logits = SOFTCAP * csum
    x_t = SOFTCAP * xt

    t = np.asarray(targets)
    vf = (t != IGNORE).astype(np.float64)
    n_valid = max(vf.sum(), 1.0)
    nll = lse - x_t
    smooth = lse - sum_logits / V
    row = (1.0 - SMOOTH) * nll + SMOOTH * smooth
    loss = (row * vf).sum() / n_valid + ZW * ((lse * vf) ** 2).sum() / n_valid
    return np.asarray(loss, dtype=np.float32)


_NC_CACHE = {}


def get_nc(dtype=DTYPE):
    if dtype not in _NC_CACHE:
        _NC_CACHE[dtype] = build_nc(dtype=dtype)
    return _NC_CACHE[dtype]


def kernel(hidden, weight, targets):
    nc = get_nc()
    in_maps = prep_inputs(hidden, weight, targets)
    res = run_bass_kernel_spmd(nc, in_maps, core_ids=list(range(N_CORES)))
    return combine(
        [res.results[c]["osum"] for c in range(N_CORES)], targets, weight.shape[0]
    )
